# revision 1
# baseline (speedup 1.0000x reference)
"""Trainium2 Bass kernel for a dense transformer block (B=2, T=2048, C=1024, H=16).

Sharding: tensor-parallel attention (2 heads/core) + AllToAll + row-parallel
FFN (512 rows/core) across 8 NeuronCores. All matmuls bf16 with fp32 PSUM
accumulation. LayerNorm affine params are folded into the adjacent weight
matrices on the host.
"""

import numpy as np
import ml_dtypes

import concourse.bass as bass
import concourse.bacc as bacc
import concourse.mybir as mybir
import concourse.tile as tile
from concourse.masks import make_identity


F32 = mybir.dt.float32
BF16 = mybir.dt.bfloat16
AF = mybir.ActivationFunctionType
ALU = mybir.AluOpType

N_CORES = 8
B, T, C, H, D, FF = 2, 2048, 1024, 16, 64, 4096
R = B * T            # 4096 total rows
RS = R // N_CORES    # 512 rows per core
KT = C // 128        # 8 k-tiles of the embedding dim
SCALE = 1.0 / np.sqrt(C)
LN_EPS = 1e-5


def build_nc():
    nc = bacc.Bacc(None, target_bir_lowering=False, debug=False, num_devices=N_CORES)

    # ---- per-core inputs (host pre-laid-out) ----
    x_bf = nc.dram_tensor("x_bf", [32, 128, C], BF16, kind="ExternalInput").ap()
    x_f32 = nc.dram_tensor("x_f32", [4, 128, C], F32, kind="ExternalInput").ap()
    wq = nc.dram_tensor("wq", [128, KT, 128], BF16, kind="ExternalInput").ap()
    wk = nc.dram_tensor("wk", [128, KT, 128], BF16, kind="ExternalInput").ap()
    wv = nc.dram_tensor("wv", [128, KT, 128], BF16, kind="ExternalInput").ap()
    bqkv = nc.dram_tensor("bqkv", [128, 3], F32, kind="ExternalInput").ap()
    wo = nc.dram_tensor("wo", [128, KT, C], BF16, kind="ExternalInput").ap()
    w1 = nc.dram_tensor("w1", [128, KT, FF], BF16, kind="ExternalInput").ap()
    b1 = nc.dram_tensor("b1", [128, 32], F32, kind="ExternalInput").ap()
    w2 = nc.dram_tensor("w2", [128, FF // 128, C], BF16, kind="ExternalInput").ap()
    b2row = nc.dram_tensor("b2row", [1, C], BF16, kind="ExternalInput").ap()
    masks = nc.dram_tensor("masks", [4, 128, 512], BF16, kind="ExternalInput").ap()
    y = nc.dram_tensor("y", [4, 128, C], F32, kind="ExternalOutput").ap()

    with tile.TileContext(nc) as tc:
        with (
            tc.tile_pool(name="const", bufs=1) as const,
            tc.tile_pool(name="ps_mm", bufs=2, space="PSUM") as ps_mm,
            tc.tile_pool(name="ps_s", bufs=2, space="PSUM") as ps_s,
            tc.tile_pool(name="ps_a", bufs=1, space="PSUM") as ps_a,
            tc.tile_pool(name="dram", bufs=1, space="DRAM") as dram,
        ):
            # constants
            ones128 = const.tile([128, 1], BF16)
            nc.any.memset(ones128[:], 1.0)
            ones64 = const.tile([1, 64], BF16)
            nc.any.memset(ones64[:], 1.0)
            epst = const.tile([128, 1], F32)
            nc.any.memset(epst[:], LN_EPS)
            mask_sb = const.tile([128, 4, 512], BF16)
            for d in range(4):
                nc.sync.dma_start(mask_sb[:, d, :], masks[d])
            wq_sb = const.tile([128, KT, 128], BF16)
            nc.sync.dma_start(wq_sb[:], wq[:])
            wk_sb = const.tile([128, KT, 128], BF16)
            nc.sync.dma_start(wk_sb[:], wk[:])
            wv_sb = const.tile([128, KT, 128], BF16)
            nc.sync.dma_start(wv_sb[:], wv[:])
            bqkv_sb = const.tile([128, 3], F32)
            nc.sync.dma_start(bqkv_sb[:], bqkv[:])
            wo_sb = const.tile([128, KT, C], BF16)
            b1_sb = const.tile([128, 32], F32)
            nc.sync.dma_start(b1_sb[:], b1[:])
            b2row_sb = const.tile([1, C], BF16)
            nc.sync.dma_start(b2row_sb[:], b2row[:])
            ones1x128 = const.tile([1, 128], BF16)
            nc.any.memset(ones1x128[:], 1.0)
            ident = const.tile([128, 128], BF16)
            make_identity(nc, ident[:])

            # HAM warmup: ~10us of PE activity with no DMA dependency
            for wi in range(48):
                ps_w = ps_mm.tile([128, 512], F32, tag="psmm")
                nc.tensor.matmul(ps_w[:, 0:128], lhsT=ident[:], rhs=ident[:],
                                 start=True, stop=True)


            a2a_in = dram.tile([N_CORES, 128, RS], BF16)
            a2a_out = dram.tile([N_CORES, 128, RS], BF16)

            # attention-stage persistent tiles
            attn_pool_cm = tc.tile_pool(name="attn", bufs=1)
            attn = attn_pool_cm.__enter__()
            qt_sb = attn.tile([128, R], BF16)   # Q^T, feature-major (2 heads stacked)
            kt_sb = attn.tile([128, R], BF16)   # K^T
            vt_sb = attn.tile([128, R], BF16)   # V^T (pre-transpose)
            v_sb = attn.tile([128, 32, 128], BF16)  # V token-major chunks
            at_sb = attn.tile([128, R], BF16)   # normalized attn out (2 heads), feature-major
            ptp_cm = tc.tile_pool(name="ptp", bufs=26)
            ptp = ptp_cm.__enter__()
            smp_cm = tc.tile_pool(name="smp", bufs=4)
            smp = smp_cm.__enter__()

            # ====== Stage A+B interleaved: LN1 + transpose + QKV per 512-row chunk ======
            with (
                tc.tile_pool(name="lnp", bufs=6) as lnp,
                tc.tile_pool(name="h1tp", bufs=3) as h1tp,
            ):
                for n in range(R // 512):
                    h1tn = h1tp.tile([128, KT, 512], BF16, tag="h1tn")
                    for i4 in range(4):
                        i = 4 * n + i4
                        xt = lnp.tile([128, C], BF16)
                        nc.sync.dma_start(xt[:], x_bf[i])
                        stats = lnp.tile([128, 2, 6], F32)
                        xr = xt[:].rearrange("p (s f) -> p s f", f=512)
                        for s in range(2):
                            nc.vector.bn_stats(out=stats[:, s, :], in_=xr[:, s, :])
                        mv = lnp.tile([128, 2], F32)
                        nc.vector.bn_aggr(out=mv[:], in_=stats[:])
                        rstd = lnp.tile([128, 1], F32)
                        nc.scalar.activation(out=rstd[:], in_=mv[:, 1:2], func=AF.Sqrt,
                                             bias=epst[:], scale=1.0)
                        nc.vector.reciprocal(out=rstd[:], in_=rstd[:])
                        hn = lnp.tile([128, C], BF16)
                        nc.vector.tensor_scalar(out=hn[:], in0=xt[:],
                                                scalar1=mv[:, 0:1], scalar2=rstd[:],
                                                op0=ALU.subtract, op1=ALU.mult)
                        for half in range(2):
                            ps_t = ps_mm.tile([128, 512], BF16, tag="psmm")
                            for j4 in range(4):
                                j = 4 * half + j4
                                nc.tensor.transpose(ps_t[:, 128 * j4:128 * (j4 + 1)],
                                                    hn[:, 128 * j:128 * (j + 1)], ident[:])
                            nc.scalar.copy(
                                out=h1tn[:, 4 * half:4 * half + 4, 128 * i4:128 * (i4 + 1)],
                                in_=ps_t[:].rearrange("p (a b) -> p a b", a=4))
                    for w_sb, out_sb, bcol in ((wq_sb, qt_sb, 0), (wk_sb, kt_sb, 1),
                                               (wv_sb, vt_sb, 2)):
                        ps = ps_mm.tile([128, 512], F32, tag="psmm")
                        for k in range(KT):
                            nc.tensor.matmul(ps[:], lhsT=w_sb[:, k, :],
                                             rhs=h1tn[:, k, :],
                                             start=(k == 0), stop=(k == KT - 1))
                        nc.scalar.activation(out=out_sb[:, 512 * n:512 * (n + 1)],
                                             in_=ps[:], func=AF.Identity,
                                             bias=bqkv_sb[:, bcol:bcol + 1], scale=1.0)
            nc.sync.dma_start(wo_sb[:], wo[:])

            # V token-major (PE transposes)
            for g in range(8):
                ps_t = ps_mm.tile([128, 512], BF16, tag="psmm")
                for j4 in range(4):
                    j = 4 * g + j4
                    nc.tensor.transpose(ps_t[:, 128 * j4:128 * (j4 + 1)],
                                        vt_sb[:, 128 * j:128 * (j + 1)], ident[:])
                nc.scalar.copy(out=v_sb[:, 4 * g:4 * g + 4, :],
                               in_=ps_t[:].rearrange("p (a b) -> p a b", a=4))

            # =============== Stage C: attention (S^T orientation) ===============
            if True:
                for b in range(B):
                    for qc in range(4):
                        q0 = b * T + 512 * qc
                        nkt = 4 * (qc + 1)
                        pts = {}
                        pa = ps_a.tile([128, 512], F32, tag="pa")
                        pasum = ps_a.tile([128, 512], F32, tag="pasum")

                        def emit_qk(k):
                            ps = ps_s.tile([128, 1024], F32, tag="pss")
                            for h in range(2):
                                hp = 64 * h
                                nc.tensor.matmul(
                                    ps[:, 512 * h:512 * (h + 1)],
                                    lhsT=kt_sb[hp:hp + 64,
                                               b * T + 128 * k:b * T + 128 * (k + 1)],
                                    rhs=qt_sb[hp:hp + 64, q0:q0 + 512],
                                    start=True, stop=True, tile_position=(hp, 0))
                            pt = ptp.tile([128, 1024], BF16, tag="pt")
                            nc.scalar.activation(out=pt[:], in_=ps[:],
                                                 func=AF.Exp, scale=SCALE)
                            if k >= 4 * qc:
                                for h in range(2):
                                    sl = pt[:, 512 * h:512 * (h + 1)]
                                    nc.vector.tensor_tensor(
                                        out=sl, in0=sl,
                                        in1=mask_sb[:, k - 4 * qc, :], op=ALU.mult)
                            pts[k] = pt

                        def emit_pv(k):
                            for h in range(2):
                                rhs_pt = pts[k][:, 512 * h:512 * (h + 1)]
                                nc.tensor.matmul(pa[64 * h:64 * h + 64, :],
                                                 lhsT=v_sb[:, b * 16 + k, 64 * h:64 * h + 64],
                                                 rhs=rhs_pt,
                                                 start=(k == 0), stop=(k == nkt - 1),
                                                 tile_position=(0, 64 * h),
                                                 skip_group_check=(h == 1))
                            for h in range(2):
                                rhs_pt = pts[k][:, 512 * h:512 * (h + 1)]
                                nc.tensor.matmul(pasum[64 * h:64 * h + 1, :],
                                                 lhsT=ones128[:],
                                                 rhs=rhs_pt,
                                                 start=(k == 0), stop=(k == nkt - 1),
                                                 tile_position=(0, 64 * h),
                                                 skip_group_check=(h == 1))

                        for k in range(nkt + 2):
                            if k < nkt:
                                emit_qk(k)
                            if k >= 2:
                                emit_pv(k - 2)
                        anums, recs = [], []
                        for h in range(2):
                            hp = 64 * h
                            anum = smp.tile([64, 512], BF16, tag="anum")
                            nc.scalar.copy(out=anum[:], in_=pa[hp:hp + 64, :])
                            anums.append(anum)
                            rec = smp.tile([1, 512], BF16, tag="rec")
                            with nc.allow_low_precision(reason="softmax denom bf16"):
                                nc.vector.reciprocal(out=rec[:], in_=pasum[hp:hp + 1, :])
                            recs.append(rec)
                        for h in range(2):
                            hp = 64 * h
                            pb = ps_mm.tile([64, 512], F32, tag="psmm")
                            nc.tensor.matmul(pb[:], lhsT=ones64[:], rhs=recs[h][:],
                                             start=True, stop=True)
                            nc.vector.tensor_tensor(out=at_sb[hp:hp + 64, q0:q0 + 512],
                                                    in0=anums[h][:], in1=pb[:], op=ALU.mult)
                        shard = b * 4 + qc
                        nc.sync.dma_start(out=a2a_in[shard],
                                          in_=at_sb[:, q0:q0 + 512])

            # =============== Stage D: AllToAll ===============
            nc.gpsimd.collective_compute(
                "AllToAll", ALU.bypass,
                replica_groups=[list(range(N_CORES))],
                ins=[a2a_in[:].opt()], outs=[a2a_out[:].opt()],
            )
            smp_cm.__exit__(None, None, None)
            ptp_cm.__exit__(None, None, None)
            attn_pool_cm.__exit__(None, None, None)

            with tc.tile_pool(name="ef", bufs=1) as ef, \
                 tc.tile_pool(name="efw", bufs=3) as efw:
                # =============== Stage E: Wo (token-major out) + residual + LN2 ===============
                w2t = ef.tile([128, 32, C], BF16, tag="w2t")
                nc.sync.dma_start(w2t[:], w2[:])
                attnt = ef.tile([128, KT, RS], BF16)
                for s in range(N_CORES):
                    nc.sync.dma_start(out=attnt[:, s, :], in_=a2a_out[s])
                x2 = ef.tile([128, 4, C], F32)
                h2t = ef.tile([128, KT, RS], BF16)
                for j in range(4):
                    xs = efw.tile([128, C], F32, tag="xs")
                    nc.sync.dma_start(xs[:], x_f32[j])
                    for cc in range(2):
                        ps = ps_mm.tile([128, 512], F32, tag="psmm")
                        for k in range(KT):
                            nc.tensor.matmul(
                                ps[:],
                                lhsT=attnt[:, k, 128 * j:128 * (j + 1)],
                                rhs=wo_sb[:, k, 512 * cc:512 * (cc + 1)],
                                start=(k == 0), stop=(k == KT - 1))
                        nc.vector.tensor_tensor(
                            out=x2[:, j, 512 * cc:512 * (cc + 1)],
                            in0=xs[:, 512 * cc:512 * (cc + 1)], in1=ps[:], op=ALU.add)
                    stats2 = efw.tile([128, 2, 6], F32, tag="st2")
                    x2r = x2[:, j, :].rearrange("p (s f) -> p s f", f=512)
                    for s in range(2):
                        nc.vector.bn_stats(out=stats2[:, s, :], in_=x2r[:, s, :])
                    mv2 = efw.tile([128, 2], F32, tag="mv2")
                    nc.vector.bn_aggr(out=mv2[:], in_=stats2[:])
                    rstd2 = efw.tile([128, 1], F32, tag="rstd2")
                    nc.scalar.activation(out=rstd2[:], in_=mv2[:, 1:2], func=AF.Sqrt,
                                         bias=epst[:], scale=1.0)
                    nc.vector.reciprocal(out=rstd2[:], in_=rstd2[:])
                    h2 = efw.tile([128, C], BF16, tag="h2")
                    nc.vector.tensor_scalar(out=h2[:], in0=x2[:, j, :],
                                            scalar1=mv2[:, 0:1], scalar2=rstd2[:],
                                            op0=ALU.subtract, op1=ALU.mult)
                    for half in range(2):
                        ps_t = ps_mm.tile([128, 512], BF16, tag="psmm")
                        for k4 in range(4):
                            k = 4 * half + k4
                            nc.tensor.transpose(ps_t[:, 128 * k4:128 * (k4 + 1)],
                                                h2[:, 128 * k:128 * (k + 1)], ident[:])
                        nc.scalar.copy(
                            out=h2t[:, 4 * half:4 * half + 4, 128 * j:128 * (j + 1)],
                            in_=ps_t[:].rearrange("p (a b) -> p a b", a=4))

                # =============== Stage F: FFN ===============
                hid = ef.tile([128, 32, RS], BF16)
                with tc.tile_pool(name="w1p", bufs=4) as w1p:
                    for m in range(32):
                        w1t = w1p.tile([128, KT, 128], BF16, tag="w1t")
                        nc.sync.dma_start(w1t[:], w1[:, :, 128 * m:128 * (m + 1)])
                        ps = ps_mm.tile([128, 512], F32, tag="psmm")
                        for k in range(KT):
                            nc.tensor.matmul(ps[:], lhsT=w1t[:, k, :], rhs=h2t[:, k, :],
                                             start=(k == 0), stop=(k == KT - 1))
                        nc.scalar.activation(out=hid[:, m, :], in_=ps[:], func=AF.Relu,
                                             bias=b1_sb[:, m:m + 1], scale=1.0)
                if True:
                    for j in range(4):
                        for cc in range(2):
                            ps = ps_mm.tile([128, 512], F32, tag="psmm")
                            for k in range(32):
                                nc.tensor.matmul(
                                    ps[:],
                                    lhsT=hid[:, k, 128 * j:128 * (j + 1)],
                                    rhs=w2t[:, k, 512 * cc:512 * (cc + 1)],
                                    start=(k == 0), stop=False)
                            nc.tensor.matmul(
                                ps[:], lhsT=ones1x128[:],
                                rhs=b2row_sb[:, 512 * cc:512 * (cc + 1)],
                                start=False, stop=True)
                            yt = efw.tile([128, 512], F32, tag="yt")
                            nc.vector.tensor_tensor(
                                out=yt[:], in0=x2[:, j, 512 * cc:512 * (cc + 1)],
                                in1=ps[:], op=ALU.add)
                            nc.sync.dma_start(y[j][:, 512 * cc:512 * (cc + 1)], yt[:])

    nc.compile()
    return nc


def prep_inputs(x, Wq, Wk, Wv, Wo, bo, W1, b1, W2, b2, g1, be1, g2, be2):
    """Host-side sharding / layout prep. Returns list of per-core input dicts."""
    bf = ml_dtypes.bfloat16
    x = np.asarray(x, np.float32).reshape(R, C)
    g1 = np.asarray(g1, np.float32); be1 = np.asarray(be1, np.float32)
    g2 = np.asarray(g2, np.float32); be2 = np.asarray(be2, np.float32)
    Wq = np.asarray(Wq, np.float32); Wk = np.asarray(Wk, np.float32)
    Wv = np.asarray(Wv, np.float32); Wo = np.asarray(Wo, np.float32)
    W1 = np.asarray(W1, np.float32); W2 = np.asarray(W2, np.float32)
    bo = np.asarray(bo, np.float32); b1 = np.asarray(b1, np.float32)
    b2 = np.asarray(b2, np.float32)

    Wq_f = g1[:, None] * Wq; bq_f = be1 @ Wq
    Wk_f = g1[:, None] * Wk; bk_f = be1 @ Wk
    Wv_f = g1[:, None] * Wv; bv_f = be1 @ Wv
    W1_f = g2[:, None] * W1; b1_f = b1 + be2 @ W1

    def lhsT_layout(w):  # [C_in, M] -> [128, C_in//128, M]
        ci, m = w.shape
        return np.ascontiguousarray(
            w.reshape(ci // 128, 128, m).transpose(1, 0, 2)).astype(bf)

    def bias_layout(v):  # [M] -> [128, M//128]
        return np.ascontiguousarray(v.reshape(-1, 128).T).astype(np.float32)

    x_bf_full = np.ascontiguousarray(x.reshape(32, 128, C)).astype(bf)
    wo_l = lhsT_layout(Wo)
    w1_l = lhsT_layout(W1_f)
    w2_l = lhsT_layout(W2)
    b1_l = bias_layout(b1_f)
    b2row = np.ascontiguousarray(b2.reshape(1, C)).astype(bf)

    # causal partial-tile masks: mask[d][kl, ql] = 1 if 128*d + kl <= ql
    masks = np.zeros((4, 128, 512), np.float32)
    for d in range(4):
        kl = 128 * d + np.arange(128)[:, None]
        ql = np.arange(512)[None, :]
        masks[d] = (kl <= ql).astype(np.float32)
    masks = masks.astype(bf)

    ins = []
    for c in range(N_CORES):
        cs = slice(128 * c, 128 * (c + 1))
        ins.append({
            "x_bf": x_bf_full,
            "x_f32": np.ascontiguousarray(
                (x[RS * c:RS * (c + 1)] + bo[None, :]).reshape(4, 128, C)
            ).astype(np.float32),
            "wq": lhsT_layout(Wq_f[:, cs]),
            "wk": lhsT_layout(Wk_f[:, cs]),
            "wv": lhsT_layout(Wv_f[:, cs]),
            "bqkv": np.ascontiguousarray(
                np.stack([bq_f[cs], bk_f[cs], bv_f[cs]], axis=1)).astype(np.float32),
            "wo": wo_l,
            "w1": w1_l, "b1": b1_l,
            "w2": w2_l, "b2row": b2row,
            "masks": masks,
        })
    return ins


_NC_CACHE = {}


def kernel(**inputs):
    import time
    from concourse.bass_utils import run_bass_kernel_spmd
    if "nc" not in _NC_CACHE:
        _NC_CACHE["nc"] = build_nc()
    nc = _NC_CACHE["nc"]
    ins = prep_inputs(**inputs)
    res = None
    last_exc = None
    for _attempt in range(4):
        try:
            res = run_bass_kernel_spmd(nc, ins, core_ids=list(range(N_CORES)))
            break
        except Exception as e:  # transient device wedge (NRT_EXEC_UNIT_UNRECOVERABLE)
            last_exc = e
            time.sleep(2)
    if res is None:
        raise last_exc
    out = np.concatenate([r["y"].reshape(RS, C) for r in res.results], axis=0)
    return out.reshape(B, T, C).astype(np.float32)



# revision 9
# speedup vs baseline: 1.3120x; 1.3120x over previous
"""Trainium2 Bass kernel for a dense transformer block (B=2, T=2048, C=1024, H=16).

Sharding v2: (batch, head-group) tensor-parallel attention across 8 cores
(core = one batch x 4 heads), AllToAll within each group of 4, then
row-parallel FFN (512 tokens/core). Everything runs feature-major off a
host-transposed x^T: LN1/LN2 stats via ones-matmuls, normalize on DVE,
no PE transposes except V. Output is y^T, un-transposed on the host.
"""

import numpy as np
import ml_dtypes

import concourse.bass as bass
import concourse.bacc as bacc
import concourse.mybir as mybir
import concourse.tile as tile
from concourse.masks import make_identity


F32 = mybir.dt.float32
BF16 = mybir.dt.bfloat16
AF = mybir.ActivationFunctionType
ALU = mybir.AluOpType

N_CORES = 8
NG = 4                  # cores per group (one batch per group)
B, T, C, H, D, FF = 2, 2048, 1024, 16, 64, 4096
HPC = H // NG           # 4 heads per core
FPC = HPC * D           # 256 features per core
KT = C // 128           # 8 k-tiles of embedding dim
CH = 512                # token chunk
NCH = T // CH           # 4 chunks per batch
SCALE = 1.0 / np.sqrt(C)
LN_EPS = 1e-5
N_WARM = 36
N_DUMMY = 44


def build_nc():
    nc = bacc.Bacc(None, target_bir_lowering=False, debug=False, num_devices=N_CORES)

    # ---- per-core inputs (host pre-laid-out) ----
    xt = nc.dram_tensor("xt", [128, KT, T], BF16, kind="ExternalInput").ap()
    xself = nc.dram_tensor("xself", [128, KT, CH], BF16, kind="ExternalInput").ap()
    wq = nc.dram_tensor("wq", [128, KT, FPC], BF16, kind="ExternalInput").ap()
    wk = nc.dram_tensor("wk", [128, KT, FPC], BF16, kind="ExternalInput").ap()
    wv = nc.dram_tensor("wv", [128, KT, FPC], BF16, kind="ExternalInput").ap()
    bqkv = nc.dram_tensor("bqkv", [128, 2, 3], F32, kind="ExternalInput").ap()
    wo = nc.dram_tensor("wo", [128, KT, KT, 128], BF16, kind="ExternalInput").ap()
    w1 = nc.dram_tensor("w1", [128, KT, FF], BF16, kind="ExternalInput").ap()
    b1 = nc.dram_tensor("b1", [128, 32], F32, kind="ExternalInput").ap()
    w2 = nc.dram_tensor("w2", [KT, 128, 32, 128], BF16, kind="ExternalInput").ap()
    b2row = nc.dram_tensor("b2row", [1, C], BF16, kind="ExternalInput").ap()
    masks = nc.dram_tensor("masks", [128, 4, 2, CH], BF16, kind="ExternalInput").ap()
    y = nc.dram_tensor("y", [KT, 128, CH], F32, kind="ExternalOutput").ap()

    with tile.TileContext(nc) as tc:
        with (
            tc.tile_pool(name="const", bufs=1) as const,
            tc.tile_pool(name="dram", bufs=1, space="DRAM") as dram,
        ):
            # constants
            ident = const.tile([128, 128], BF16)
            make_identity(nc, ident[:])
            ones_c = const.tile([128, 1], BF16)
            nc.any.memset(ones_c[:], 1.0 / C)
            ones_1 = const.tile([128, 1], BF16)
            nc.any.memset(ones_1[:], 1.0)
            ones64b = const.tile([128, 64], BF16)
            nc.any.memset(ones64b[:], 1.0)
            onesrow = const.tile([1, CH], BF16)
            nc.any.memset(onesrow[:], 1.0)
            eps1 = const.tile([1, 1], F32)
            nc.any.memset(eps1[:], LN_EPS)

            wq_sb = const.tile([128, KT, FPC], BF16)
            nc.sync.dma_start(wq_sb[:], wq[:])
            wk_sb = const.tile([128, KT, FPC], BF16)
            nc.sync.dma_start(wk_sb[:], wk[:])
            wv_sb = const.tile([128, KT, FPC], BF16)
            nc.sync.dma_start(wv_sb[:], wv[:])
            bqkv_sb = const.tile([128, 2, 3], F32)
            nc.sync.dma_start(bqkv_sb[:], bqkv[:])
            xself_sb = const.tile([128, KT, CH], BF16)
            nc.sync.dma_start(xself_sb[:], xself[:])
            wo_sb = const.tile([128, KT, KT, 128], BF16)
            b1_sb = const.tile([128, 32], F32)
            nc.sync.dma_start(b1_sb[:], b1[:])
            b2row_sb = const.tile([1, C], BF16)
            nc.sync.dma_start(b2row_sb[:], b2row[:])

            # A2A slots: dest core c' gets my 4 heads for a 256-token strip of
            # my batch (stage E rows are cross-batch: 256 from b0 + 256 from b1)
            a2a_in = dram.tile([N_CORES, 2, 128, CH // 2], BF16)
            a2a_out = dram.tile([N_CORES, 2, 128, CH // 2], BF16)

            # attention persistent tensors
            qkv_cm = tc.tile_pool(name="qkvp", bufs=1)
            qkvp = qkv_cm.__enter__()
            qt_sb = qkvp.tile([128, 2, T], BF16)
            kt_sb = qkvp.tile([128, 2, T], BF16)
            vt_sb = qkvp.tile([128, 2, T], BF16)
            v_sb = qkvp.tile([128, T // 128, FPC], BF16)
            masks_sb = qkvp.tile([128, 4, 2, CH], BF16)
            nc.sync.dma_start(masks_sb[:], masks[:])

            # ================= Phase 1: LN1 + QKV (feature-major) =================
            with (
                tc.tile_pool(name="p1", bufs=2) as p1,
                tc.tile_pool(name="p1s", bufs=2) as p1s,
                tc.tile_pool(name="ps1", bufs=3, space="PSUM") as ps1,
                tc.tile_pool(name="pstat", bufs=2, space="PSUM") as pstat,
            ):
                # HAM warmup: PE activity with no DMA dependency
                ps_w = ps1.tile([128, CH], F32, tag="warm", bufs=1)
                for wi in range(N_WARM):
                    nc.tensor.matmul(ps_w[:, 0:128], lhsT=ident[:], rhs=ident[:],
                                     start=(wi == 0), stop=(wi == N_WARM - 1))

                for n in range(NCH):
                    q0 = CH * n
                    xt_c = p1.tile([128, KT, CH], BF16, tag="xt")
                    nc.sync.dma_start(xt_c[:], xt[:, :, q0:q0 + CH])
                    st = pstat.tile([128, CH], F32, tag="stat")
                    for k in range(KT):
                        nc.tensor.matmul(st[0:1, :], lhsT=ones_c[:], rhs=xt_c[:, k, :],
                                         start=(k == 0), stop=(k == KT - 1))
                    mur = p1s.tile([1, CH], BF16, tag="mur")
                    nc.scalar.copy(out=mur[:], in_=st[0:1, :])
                    psb = ps1.tile([128, CH], F32, tag="psmm")
                    nc.tensor.matmul(psb[:], lhsT=onesrow[0:1, 0:128], rhs=mur[:],
                                     start=True, stop=True)
                    mub = p1s.tile([128, CH], BF16, tag="mub")
                    nc.scalar.copy(out=mub[:], in_=psb[:])
                    xc = p1.tile([128, KT, CH], BF16, tag="xc")
                    for k in range(KT):
                        nc.vector.tensor_tensor(out=xc[:, k, :], in0=xt_c[:, k, :],
                                                in1=mub[:], op=ALU.subtract)
                    for k in range(KT):
                        sq = p1s.tile([128, CH], BF16, tag="sq", bufs=3)
                        nc.vector.tensor_tensor(out=sq[:], in0=xc[:, k, :],
                                                in1=xc[:, k, :], op=ALU.mult)
                        nc.tensor.matmul(st[32:33, :], lhsT=ones_c[:], rhs=sq[:],
                                         start=(k == 0), stop=(k == KT - 1))
                    stdr = p1s.tile([1, CH], F32, tag="stdr")
                    nc.scalar.activation(out=stdr[:], in_=st[32:33, :], func=AF.Sqrt,
                                         bias=eps1[:], scale=1.0)
                    rstdr = p1s.tile([1, CH], BF16, tag="rstdr")
                    with nc.allow_low_precision(reason="ln rstd bf16"):
                        nc.vector.reciprocal(out=rstdr[:], in_=stdr[:])
                    psb2 = ps1.tile([128, CH], F32, tag="psmm")
                    nc.tensor.matmul(psb2[:], lhsT=onesrow[0:1, 0:128], rhs=rstdr[:],
                                     start=True, stop=True)
                    rstdb = p1s.tile([128, CH], BF16, tag="rstdb")
                    nc.scalar.copy(out=rstdb[:], in_=psb2[:])
                    h_c = p1.tile([128, KT, CH], BF16, tag="h")
                    for k in range(KT):
                        nc.vector.tensor_tensor(out=h_c[:, k, :], in0=xc[:, k, :],
                                                in1=rstdb[:], op=ALU.mult)
                    # QKV matmuls (feature-major out)
                    for w_sb, out_sb, col in ((wq_sb, qt_sb, 0), (wk_sb, kt_sb, 1),
                                              (wv_sb, vt_sb, 2)):
                        for g in range(2):
                            ps = ps1.tile([128, CH], F32, tag="psmm")
                            for k in range(KT):
                                nc.tensor.matmul(ps[:], lhsT=w_sb[:, k, 128 * g:128 * (g + 1)],
                                                 rhs=h_c[:, k, :],
                                                 start=(k == 0), stop=(k == KT - 1))
                            nc.vector.tensor_scalar(
                                out=out_sb[:, g, q0:q0 + CH], in0=ps[:],
                                scalar1=bqkv_sb[:, g, col:col + 1], scalar2=None,
                                op0=ALU.add)
                    # V -> token-major for this chunk (PE transposes)
                    for g in range(2):
                        ps_t = ps1.tile([128, CH], BF16, tag="psmm")
                        for u in range(4):
                            nc.tensor.transpose(
                                ps_t[:, 128 * u:128 * (u + 1)],
                                vt_sb[:, g, q0 + 128 * u:q0 + 128 * (u + 1)], ident[:])
                        nc.scalar.copy(
                            out=v_sb[:, 4 * n:4 * n + 4, 128 * g:128 * (g + 1)],
                            in_=ps_t[:].rearrange("p (a b) -> p a b", a=4))
                nc.sync.dma_start(wo_sb[:], wo[:])

            # ================= Phase 2: attention (S^T orientation) =================
            with (
                tc.tile_pool(name="pss", bufs=1, space="PSUM") as pssp,
                tc.tile_pool(name="pap", bufs=1, space="PSUM") as pap,
                tc.tile_pool(name="psr", bufs=1, space="PSUM") as psr,
                tc.tile_pool(name="ptp", bufs=6) as ptp,
                tc.tile_pool(name="smp", bufs=2) as smp,
            ):
                for qc in range(NCH):
                    q0 = CH * qc
                    nkt = 4 * (qc + 1)
                    pa0 = pap.tile([128, CH], F32, tag="pa0", name="pa0")
                    pa1 = pap.tile([128, CH], F32, tag="pa1", name="pa1")
                    pa = [pa0, pa1]
                    pasum = pap.tile([128, CH], F32, tag="pasum")
                    pts = {}

                    def emit_qk(k):
                        d = k - 4 * qc
                        qlo = 128 * d if d > 0 else 0
                        for gg in range(2):
                            ps = pssp.tile([128, 2, CH], F32, tag=f"pss{gg}")
                            for hh in range(2):
                                hp = 64 * hh
                                nc.tensor.matmul(
                                    ps[:, hh, qlo:],
                                    lhsT=kt_sb[hp:hp + 64, gg, 128 * k:128 * (k + 1)],
                                    rhs=qt_sb[hp:hp + 64, gg, q0 + qlo:q0 + CH],
                                    start=True, stop=True, tile_position=(hp, 0))
                            pt = ptp.tile([128, 2, CH], BF16, tag="pt")
                            nc.scalar.activation(out=pt[:, :, qlo:], in_=ps[:, :, qlo:],
                                                 func=AF.Exp, scale=SCALE)
                            if d >= 0:
                                nc.gpsimd.tensor_tensor(
                                    out=pt[:, :, qlo:], in0=pt[:, :, qlo:],
                                    in1=masks_sb[:, d, :, qlo:], op=ALU.mult)
                            pts[(k, gg)] = pt

                    def emit_pv(k):
                        d = k - 4 * qc
                        qlo = 128 * d if d > 0 else 0
                        for gg in range(2):
                            pt = pts.pop((k, gg))
                            for hh in range(2):
                                h = 2 * gg + hh
                                nc.tensor.matmul(
                                    pa[gg][64 * hh:64 * (hh + 1), qlo:],
                                    lhsT=v_sb[:, k, 128 * gg + 64 * hh:128 * gg + 64 * (hh + 1)],
                                    rhs=pt[:, hh, qlo:],
                                    start=(k == 0), stop=(k == nkt - 1),
                                    tile_position=(0, 64 * hh),
                                    skip_group_check=(hh == 1))
                            for hh in range(2):
                                h = 2 * gg + hh
                                nc.tensor.matmul(
                                    pasum[32 * h:32 * h + 1, qlo:],
                                    lhsT=ones_1[:], rhs=pt[:, hh, qlo:],
                                    start=(k == 0), stop=(k == nkt - 1),
                                    tile_position=(0, 32 * h),
                                    skip_group_check=(h > 0))

                    for k in range(nkt + 2):
                        if k < nkt:
                            emit_qk(k)
                        if k >= 2:
                            emit_pv(k - 2)

                    rec = smp.tile([128, CH], BF16, tag="rec")
                    with nc.allow_low_precision(reason="softmax denom bf16"):
                        nc.vector.reciprocal(out=rec[:], in_=pasum[:])
                    for gg in range(2):
                        an = smp.tile([128, CH], BF16, tag=f"an{gg}")
                        nc.scalar.copy(out=an[:], in_=pa[gg][:])
                        rb = psr.tile([128, CH], F32, tag="recb")
                        for hh in range(2):
                            h = 2 * gg + hh
                            nc.tensor.matmul(
                                rb[64 * hh:64 * (hh + 1), :],
                                lhsT=ones64b[32 * h:32 * h + 1, :],
                                rhs=rec[32 * h:32 * h + 1, :],
                                start=True, stop=True,
                                tile_position=(32 * h, 64 * hh),
                                skip_group_check=(hh == 1))
                        at_t = smp.tile([128, CH], BF16, tag=f"at{gg}")
                        nc.vector.tensor_tensor(out=at_t[:], in0=an[:], in1=rb[:],
                                                op=ALU.mult)
                        nc.sync.dma_start(out=a2a_in[2 * qc, gg],
                                          in_=at_t[:, 0:CH // 2])
                        nc.sync.dma_start(out=a2a_in[2 * qc + 1, gg],
                                          in_=at_t[:, CH // 2:CH])

            qkv_cm.__exit__(None, None, None)

            # ================= Phase 3: AllToAll within each group of 4 ============
            nc.gpsimd.collective_compute(
                "AllToAll", ALU.bypass,
                replica_groups=[list(range(N_CORES))],
                ins=[a2a_in[:].opt()], outs=[a2a_out[:].opt()],
            )

            # ================= Phase 4: Wo + LN2 + FFN (feature-major) ============
            with (
                tc.tile_pool(name="ef", bufs=1) as ef,
                tc.tile_pool(name="efw", bufs=2) as efw,
                tc.tile_pool(name="psE", bufs=3, space="PSUM") as psE,
                tc.tile_pool(name="psES", bufs=1, space="PSUM") as psES,
                tc.tile_pool(name="w1p", bufs=4) as w1p,
                tc.tile_pool(name="w2p", bufs=3) as w2p,
            ):
                # keep-warm dummies riding over the collective
                dm = psES.tile([128, CH], F32, tag="dummy")
                for i in range(N_DUMMY):
                    nc.tensor.matmul(dm[:, 0:128], lhsT=ident[:],
                                     rhs=xself_sb[:, 0, 0:128],
                                     start=(i == 0), stop=(i == N_DUMMY - 1))

                # token axis of stage E: [0:256] = batch-0 strip, [256:512] = batch-1
                attnt = ef.tile([128, KT, CH], BF16)
                for s in range(N_CORES):
                    bs, hgs = s // NG, s % NG
                    for g in range(2):
                        nc.sync.dma_start(
                            out=attnt[:, 2 * hgs + g,
                                      (CH // 2) * bs:(CH // 2) * (bs + 1)],
                            in_=a2a_out[s, g])

                # Wo + residual (+bo folded into xself on host)
                x2 = ef.tile([128, KT, CH], BF16)
                for co in range(KT):
                    ps = psE.tile([128, CH], F32, tag="ps")
                    for kf in range(KT):
                        nc.tensor.matmul(ps[:], lhsT=wo_sb[:, kf, co, :],
                                         rhs=attnt[:, kf, :],
                                         start=(kf == 0), stop=(kf == KT - 1))
                    nc.vector.tensor_tensor(out=x2[:, co, :], in0=ps[:],
                                            in1=xself_sb[:, co, :], op=ALU.add)

                # LN2 (feature-major stats)
                st2 = psES.tile([128, CH], F32, tag="stat2")
                for k in range(KT):
                    nc.tensor.matmul(st2[0:1, :], lhsT=ones_c[:], rhs=x2[:, k, :],
                                     start=(k == 0), stop=(k == KT - 1))
                mur2 = efw.tile([1, CH], BF16, tag="mur2")
                nc.scalar.copy(out=mur2[:], in_=st2[0:1, :])
                psb3 = psE.tile([128, CH], F32, tag="ps")
                nc.tensor.matmul(psb3[:], lhsT=onesrow[0:1, 0:128], rhs=mur2[:],
                                 start=True, stop=True)
                mub2 = efw.tile([128, CH], BF16, tag="mub2")
                nc.scalar.copy(out=mub2[:], in_=psb3[:])
                xc2 = ef.tile([128, KT, CH], BF16)
                for k in range(KT):
                    nc.vector.tensor_tensor(out=xc2[:, k, :], in0=x2[:, k, :],
                                            in1=mub2[:], op=ALU.subtract)
                for k in range(KT):
                    sq2 = efw.tile([128, CH], BF16, tag="sq2", bufs=3)
                    nc.vector.tensor_tensor(out=sq2[:], in0=xc2[:, k, :],
                                            in1=xc2[:, k, :], op=ALU.mult)
                    nc.tensor.matmul(st2[32:33, :], lhsT=ones_c[:], rhs=sq2[:],
                                     start=(k == 0), stop=(k == KT - 1))
                stdr2 = efw.tile([1, CH], F32, tag="stdr2")
                nc.scalar.activation(out=stdr2[:], in_=st2[32:33, :], func=AF.Sqrt,
                                     bias=eps1[:], scale=1.0)
                rstdr2 = efw.tile([1, CH], BF16, tag="rstdr2")
                with nc.allow_low_precision(reason="ln2 rstd bf16"):
                    nc.vector.reciprocal(out=rstdr2[:], in_=stdr2[:])
                psb4 = psE.tile([128, CH], F32, tag="ps")
                nc.tensor.matmul(psb4[:], lhsT=onesrow[0:1, 0:128], rhs=rstdr2[:],
                                 start=True, stop=True)
                rstdb2 = efw.tile([128, CH], BF16, tag="rstdb2")
                nc.scalar.copy(out=rstdb2[:], in_=psb4[:])
                h2t = ef.tile([128, KT, CH], BF16)
                for k in range(KT):
                    nc.vector.tensor_tensor(out=h2t[:, k, :], in0=xc2[:, k, :],
                                            in1=rstdb2[:], op=ALU.mult)

                # FFN1
                hid = ef.tile([128, 32, CH], BF16)
                for m in range(32):
                    w1t = w1p.tile([128, KT, 128], BF16, tag="w1t")
                    nc.sync.dma_start(w1t[:], w1[:, :, 128 * m:128 * (m + 1)])
                    ps = psE.tile([128, CH], F32, tag="ps")
                    for k in range(KT):
                        nc.tensor.matmul(ps[:], lhsT=w1t[:, k, :], rhs=h2t[:, k, :],
                                         start=(k == 0), stop=(k == KT - 1))
                    nc.vector.tensor_scalar(out=hid[:, m, :], in0=ps[:],
                                            scalar1=b1_sb[:, m:m + 1], scalar2=0.0,
                                            op0=ALU.add, op1=ALU.max)

                # FFN2 (feature-major out) + residual + b2
                for co in range(KT):
                    w2t = w2p.tile([128, 32, 128], BF16, tag="w2t")
                    nc.sync.dma_start(w2t[:], w2[co])
                    ps = psE.tile([128, CH], F32, tag="ps")
                    for m in range(32):
                        nc.tensor.matmul(ps[:], lhsT=w2t[:, m, :], rhs=hid[:, m, :],
                                         start=(m == 0), stop=False)
                    nc.tensor.matmul(ps[:], lhsT=b2row_sb[0:1, 128 * co:128 * (co + 1)],
                                     rhs=onesrow[:], start=False, stop=True)
                    yt = efw.tile([128, CH], F32, tag="yt")
                    nc.vector.tensor_tensor(out=yt[:], in0=ps[:], in1=x2[:, co, :],
                                            op=ALU.add)
                    nc.sync.dma_start(y[co], yt[:])

    nc.compile()
    return nc


def prep_inputs(x, Wq, Wk, Wv, Wo, bo, W1, b1, W2, b2, g1, be1, g2, be2):
    """Host-side sharding / layout prep. Returns list of per-core input dicts."""
    bf = ml_dtypes.bfloat16
    x = np.asarray(x, np.float32)
    g1 = np.asarray(g1, np.float32); be1 = np.asarray(be1, np.float32)
    g2 = np.asarray(g2, np.float32); be2 = np.asarray(be2, np.float32)
    Wq = np.asarray(Wq, np.float32); Wk = np.asarray(Wk, np.float32)
    Wv = np.asarray(Wv, np.float32); Wo = np.asarray(Wo, np.float32)
    W1 = np.asarray(W1, np.float32); W2 = np.asarray(W2, np.float32)
    bo = np.asarray(bo, np.float32); b1 = np.asarray(b1, np.float32)
    b2 = np.asarray(b2, np.float32)

    Wq_f = g1[:, None] * Wq; bq_f = be1 @ Wq
    Wk_f = g1[:, None] * Wk; bk_f = be1 @ Wk
    Wv_f = g1[:, None] * Wv; bv_f = be1 @ Wv
    W1_f = g2[:, None] * W1; b1_f = b1 + be2 @ W1

    def lhsT_layout(w):  # [C_in, M] -> [128, C_in//128, M]
        ci, m = w.shape
        return np.ascontiguousarray(
            w.reshape(ci // 128, 128, m).transpose(1, 0, 2)).astype(bf)

    def tmajor(a):  # [rows, C] -> x^T tiles [128, KT, rows]
        return np.ascontiguousarray(
            a.T.reshape(KT, 128, -1).transpose(1, 0, 2)).astype(bf)

    wo_l = np.ascontiguousarray(
        Wo.reshape(KT, 128, KT, 128).transpose(1, 0, 2, 3)).astype(bf)
    w1_l = lhsT_layout(W1_f)
    w2_l = np.ascontiguousarray(
        W2.reshape(32, 128, KT, 128).transpose(2, 1, 0, 3)).astype(bf)
    b1_l = np.ascontiguousarray(b1_f.reshape(32, 128).T).astype(np.float32)
    b2_l = np.ascontiguousarray(b2.reshape(1, C)).astype(bf)

    # causal masks [p, d, hh, q]: valid iff 128*d + p <= q (hh dim replicated)
    m3 = np.zeros((128, 4, CH), np.float32)
    for d in range(4):
        kl = 128 * d + np.arange(128)[:, None]
        ql = np.arange(CH)[None, :]
        m3[:, d, :] = (kl <= ql).astype(np.float32)
    masks_l = np.ascontiguousarray(
        np.repeat(m3[:, :, None, :], 2, axis=2)).astype(bf)

    ins = []
    for c in range(N_CORES):
        b = c // NG
        j = c % NG
        cols = slice(FPC * j, FPC * (j + 1))
        xb = x[b]  # [T, C]
        strip = slice((CH // 2) * c, (CH // 2) * (c + 1))
        xown = np.concatenate([x[0][strip], x[1][strip]], axis=0)  # [CH, C]
        bq_c = bq_f[cols].reshape(2, 128).T
        bk_c = bk_f[cols].reshape(2, 128).T
        bv_c = bv_f[cols].reshape(2, 128).T
        ins.append({
            "xt": tmajor(xb),
            "xself": tmajor(xown + bo[None, :]),
            "wq": lhsT_layout(Wq_f[:, cols]),
            "wk": lhsT_layout(Wk_f[:, cols]),
            "wv": lhsT_layout(Wv_f[:, cols]),
            "bqkv": np.ascontiguousarray(
                np.stack([bq_c, bk_c, bv_c], axis=2)).astype(np.float32),
            "wo": wo_l,
            "w1": w1_l, "b1": b1_l,
            "w2": w2_l, "b2row": b2_l,
            "masks": masks_l,
        })
    return ins


def postprocess(results):
    """Per-core y^T tiles [KT, 128, CH] -> full [B, T, C]."""
    out = np.empty((B, T, C), np.float32)
    hs = CH // 2
    for c, r in enumerate(results):
        yt = np.asarray(r["y"], np.float32).reshape(C, CH)  # [feats, toks]
        out[0, hs * c:hs * (c + 1), :] = yt[:, 0:hs].T
        out[1, hs * c:hs * (c + 1), :] = yt[:, hs:CH].T
    return out


_NC_CACHE = {}


def kernel(**inputs):
    import time
    from concourse.bass_utils import run_bass_kernel_spmd
    if "nc" not in _NC_CACHE:
        _NC_CACHE["nc"] = build_nc()
    nc = _NC_CACHE["nc"]
    ins = prep_inputs(**inputs)
    res = None
    last_exc = None
    for _attempt in range(4):
        try:
            res = run_bass_kernel_spmd(nc, ins, core_ids=list(range(N_CORES)))
            break
        except Exception as e:  # transient device wedge
            last_exc = e
            time.sleep(2)
    if res is None:
        raise last_exc
    return postprocess(res.results)


# revision 10
# speedup vs baseline: 1.7706x; 1.3495x over previous
"""Trainium2 Bass kernel for a dense transformer block (B=2, T=2048, C=1024, H=16).

Sharding v3: (batch, head-group) tensor-parallel attention across 8 cores
(core = one batch x 4 heads), 8-rank AllToAll with cross-batch 256-token
strips, then row-parallel FFN (512 tokens/core). Feature-major dataflow off
a host-transposed x^T; LN stats via ones-matmuls; fp8 DoubleRow matmuls for
QKV / Wo / FFN with host-scaled weights; fp8 A2A payload. Output y^T is
un-transposed on the host.
"""

import numpy as np
import ml_dtypes

import concourse.bass as bass
import concourse.bacc as bacc
import concourse.mybir as mybir
import concourse.tile as tile
from concourse.masks import make_identity


F32 = mybir.dt.float32
BF16 = mybir.dt.bfloat16
F8 = mybir.dt.float8e4
AF = mybir.ActivationFunctionType
ALU = mybir.AluOpType
DR = mybir.MatmulPerfMode.DoubleRow

N_CORES = 8
NG = 4                  # cores per group (one batch per group)
B, T, C, H, D, FF = 2, 2048, 1024, 16, 64, 4096
HPC = H // NG           # 4 heads per core
FPC = HPC * D           # 256 features per core
KT = C // 128           # 8 k-tiles of embedding dim
CH = 512                # token chunk
NCH = T // CH           # 4 chunks per batch
SCALE = 1.0 / np.sqrt(C)
LN_EPS = 1e-5
SW = 2.0 ** 12          # fp8 weight scale (wq/wk/wv/wo/w1)
SW2 = 2.0 ** 13         # fp8 weight scale (w2)
ISW = 1.0 / SW
ISW2 = 1.0 / SW2
N_WARM = 36
N_DUMMY = 44


def build_nc():
    nc = bacc.Bacc(None, target_bir_lowering=False, debug=False, num_devices=N_CORES)

    # ---- per-core inputs (host pre-laid-out) ----
    xt = nc.dram_tensor("xt", [128, KT, T], BF16, kind="ExternalInput").ap()
    xself = nc.dram_tensor("xself", [128, KT, CH], BF16, kind="ExternalInput").ap()
    wq = nc.dram_tensor("wq", [128, KT, FPC], F8, kind="ExternalInput").ap()
    wk = nc.dram_tensor("wk", [128, KT, FPC], F8, kind="ExternalInput").ap()
    wv = nc.dram_tensor("wv", [128, KT, FPC], F8, kind="ExternalInput").ap()
    bqkv = nc.dram_tensor("bqkv", [128, 2, 3], F32, kind="ExternalInput").ap()
    wo = nc.dram_tensor("wo", [128, KT, KT, 128], F8, kind="ExternalInput").ap()
    bo_col = nc.dram_tensor("bo_col", [128, KT], F32, kind="ExternalInput").ap()
    w1 = nc.dram_tensor("w1", [128, KT, FF], F8, kind="ExternalInput").ap()
    b1 = nc.dram_tensor("b1", [128, 32], F32, kind="ExternalInput").ap()
    w2 = nc.dram_tensor("w2", [KT, 128, 32, 128], F8, kind="ExternalInput").ap()
    b2col = nc.dram_tensor("b2col", [128, KT], F32, kind="ExternalInput").ap()
    masks = nc.dram_tensor("masks", [128, 4, 2, CH], BF16, kind="ExternalInput").ap()
    y = nc.dram_tensor("y", [KT, 128, CH], F32, kind="ExternalOutput").ap()

    with tile.TileContext(nc) as tc:
        with (
            tc.tile_pool(name="const", bufs=1) as const,
            tc.tile_pool(name="dram", bufs=1, space="DRAM") as dram,
        ):
            ident = const.tile([128, 128], BF16)
            make_identity(nc, ident[:])
            ones_c = const.tile([128, 1], BF16)
            nc.any.memset(ones_c[:], 1.0 / C)
            ones_1 = const.tile([128, 1], BF16)
            nc.any.memset(ones_1[:], 1.0)
            ones64b = const.tile([128, 64], BF16)
            nc.any.memset(ones64b[:], 1.0)
            onesrow = const.tile([1, CH], BF16)
            nc.any.memset(onesrow[:], 1.0)
            eps1 = const.tile([1, 1], F32)
            nc.any.memset(eps1[:], LN_EPS)

            # A2A slots: dest core c' gets my 4 heads for a 256-token strip of
            # my batch (stage E rows: 256 from b0 + 256 from b1)
            a2a_in = dram.tile([N_CORES, 2, 128, CH // 2], F8)
            a2a_out = dram.tile([N_CORES, 2, 128, CH // 2], F8)

            # attention persistent tensors
            qkv_cm = tc.tile_pool(name="qkvp", bufs=1)
            qkvp = qkv_cm.__enter__()
            qt_sb = qkvp.tile([128, 2, T], BF16)
            kt_sb = qkvp.tile([128, 2, T], BF16)
            vt_sb = qkvp.tile([128, 2, T], BF16)
            v_sb = qkvp.tile([128, T // 128, FPC], BF16)
            masks_sb = qkvp.tile([128, 4, 2, CH], BF16)

            # ================= Phase 1: LN1 + QKV (feature-major) =================
            with (
                tc.tile_pool(name="p1", bufs=2) as p1,
                tc.tile_pool(name="p1s", bufs=2) as p1s,
                tc.tile_pool(name="ps1", bufs=3, space="PSUM") as ps1,
                tc.tile_pool(name="pstat", bufs=2, space="PSUM") as pstat,
            ):
                # x^T chunk loads first so the stats matmuls can start early
                xts = []
                for n in range(NCH):
                    xt_c = p1.tile([128, KT, CH], BF16, tag="xt", bufs=4,
                                   name=f"xtc{n}")
                    nc.sync.dma_start(xt_c[:], xt[:, :, CH * n:CH * (n + 1)])
                    xts.append(xt_c)
                # weights after the x^T stream
                wq_sb = const.tile([128, KT, FPC], F8, name="wq_sb")
                nc.sync.dma_start(wq_sb[:], wq[:])
                wk_sb = const.tile([128, KT, FPC], F8, name="wk_sb")
                nc.sync.dma_start(wk_sb[:], wk[:])
                wv_sb = const.tile([128, KT, FPC], F8, name="wv_sb")
                nc.sync.dma_start(wv_sb[:], wv[:])
                bqkv_sb = const.tile([128, 2, 3], F32, name="bqkv_sb")
                nc.sync.dma_start(bqkv_sb[:], bqkv[:])
                nc.sync.dma_start(masks_sb[:], masks[:])
                xself_sb = const.tile([128, KT, CH], BF16, name="xself_sb")
                nc.sync.dma_start(xself_sb[:], xself[:])
                wo_sb = const.tile([128, KT, KT, 128], F8, name="wo_sb")
                bo_sb = const.tile([128, KT], F32, name="bo_sb")
                nc.sync.dma_start(bo_sb[:], bo_col[:])
                b1_sb = const.tile([128, 32], F32, name="b1_sb")
                nc.sync.dma_start(b1_sb[:], b1[:])
                b2_sb = const.tile([128, KT], F32, name="b2_sb")
                nc.sync.dma_start(b2_sb[:], b2col[:])

                # HAM warmup: PE activity with no DMA dependency
                ps_w = ps1.tile([128, CH], F32, tag="warm", bufs=1)
                for wi in range(N_WARM):
                    nc.tensor.matmul(ps_w[:, 0:128], lhsT=ident[:], rhs=ident[:],
                                     start=(wi == 0), stop=(wi == N_WARM - 1))

                for n in range(NCH):
                    q0 = CH * n
                    xt_c = xts[n]
                    st = pstat.tile([128, CH], F32, tag="stat")
                    for k in range(KT):
                        nc.tensor.matmul(st[0:1, :], lhsT=ones_c[:], rhs=xt_c[:, k, :],
                                         start=(k == 0), stop=(k == KT - 1))
                    # E[x^2] from raw x (independent of mu -> shorter chain)
                    for k in range(KT):
                        sq = p1s.tile([128, CH], BF16, tag="sq", bufs=3)
                        nc.vector.tensor_tensor(out=sq[:], in0=xt_c[:, k, :],
                                                in1=xt_c[:, k, :], op=ALU.mult)
                        nc.tensor.matmul(st[32:33, :], lhsT=ones_c[:], rhs=sq[:],
                                         start=(k == 0), stop=(k == KT - 1))
                    mur = p1s.tile([1, CH], BF16, tag="mur")
                    nc.scalar.copy(out=mur[:], in_=st[0:1, :])
                    murf = p1s.tile([1, CH], F32, tag="murf")
                    nc.scalar.copy(out=murf[:], in_=st[0:1, :])
                    psb = ps1.tile([128, CH], F32, tag="psmm")
                    nc.tensor.matmul(psb[:], lhsT=onesrow[0:1, 0:128], rhs=mur[:],
                                     start=True, stop=True)
                    mub = p1s.tile([128, CH], BF16, tag="mub")
                    nc.scalar.copy(out=mub[:], in_=psb[:])
                    musq = p1s.tile([1, CH], F32, tag="musq")
                    nc.vector.tensor_tensor(out=musq[:], in0=murf[:], in1=murf[:],
                                            op=ALU.mult)
                    varr = p1s.tile([1, CH], F32, tag="varr")
                    nc.vector.tensor_tensor(out=varr[:], in0=st[32:33, :], in1=musq[:],
                                            op=ALU.subtract)
                    stdr = p1s.tile([1, CH], F32, tag="stdr")
                    nc.scalar.activation(out=stdr[:], in_=varr[:], func=AF.Sqrt,
                                         bias=eps1[:], scale=1.0)
                    rstdr = p1s.tile([1, CH], BF16, tag="rstdr")
                    with nc.allow_low_precision(reason="ln rstd bf16"):
                        nc.vector.reciprocal(out=rstdr[:], in_=stdr[:])
                    psb2 = ps1.tile([128, CH], F32, tag="psmm")
                    nc.tensor.matmul(psb2[:], lhsT=onesrow[0:1, 0:128], rhs=rstdr[:],
                                     start=True, stop=True)
                    rstdb = p1s.tile([128, CH], BF16, tag="rstdb")
                    nc.scalar.copy(out=rstdb[:], in_=psb2[:])
                    xc = p1.tile([128, KT, CH], BF16, tag="xc")
                    for k in range(KT):
                        nc.vector.tensor_tensor(out=xc[:, k, :], in0=xt_c[:, k, :],
                                                in1=mub[:], op=ALU.subtract)
                    h_c = p1.tile([128, KT, CH], F8, tag="h")
                    for k in range(KT):
                        nc.vector.tensor_tensor(out=h_c[:, k, :], in0=xc[:, k, :],
                                                in1=rstdb[:], op=ALU.mult)
                    # QKV matmuls: fp8 DoubleRow, weights pre-scaled by SW
                    for w_sb, out_sb, col in ((wq_sb, qt_sb, 0), (wk_sb, kt_sb, 1),
                                              (wv_sb, vt_sb, 2)):
                        for g in range(2):
                            ps = ps1.tile([128, CH], F32, tag="psmm")
                            for t2 in range(KT // 2):
                                nc.tensor.matmul(
                                    ps[:],
                                    lhsT=w_sb[:, 2 * t2:2 * t2 + 2, 128 * g:128 * (g + 1)],
                                    rhs=h_c[:, 2 * t2:2 * t2 + 2, :],
                                    start=(t2 == 0), stop=(t2 == KT // 2 - 1),
                                    perf_mode=DR)
                            nc.vector.tensor_scalar(
                                out=out_sb[:, g, q0:q0 + CH], in0=ps[:],
                                scalar1=ISW, scalar2=bqkv_sb[:, g, col:col + 1],
                                op0=ALU.mult, op1=ALU.add)
                    # V -> token-major for this chunk (PE transposes)
                    for g in range(2):
                        ps_t = ps1.tile([128, CH], BF16, tag="psmm")
                        for u in range(4):
                            nc.tensor.transpose(
                                ps_t[:, 128 * u:128 * (u + 1)],
                                vt_sb[:, g, q0 + 128 * u:q0 + 128 * (u + 1)], ident[:])
                        nc.scalar.copy(
                            out=v_sb[:, 4 * n:4 * n + 4, 128 * g:128 * (g + 1)],
                            in_=ps_t[:].rearrange("p (a b) -> p a b", a=4))
                nc.sync.dma_start(wo_sb[:], wo[:])

            # ================= Phase 2: attention (S^T orientation) =================
            with (
                tc.tile_pool(name="pss", bufs=1, space="PSUM") as pssp,
                tc.tile_pool(name="pap", bufs=1, space="PSUM") as pap,
                tc.tile_pool(name="psr", bufs=1, space="PSUM") as psr,
                tc.tile_pool(name="ptp", bufs=9) as ptp,
                tc.tile_pool(name="smp", bufs=2) as smp,
            ):
                for qc in range(NCH):
                    q0 = CH * qc
                    nkt = 4 * (qc + 1)
                    pa0 = pap.tile([128, CH], F32, tag="pa0", name="pa0")
                    pa1 = pap.tile([128, CH], F32, tag="pa1", name="pa1")
                    pa = [pa0, pa1]
                    pasum = pap.tile([128, CH], F32, tag="pasum")
                    pts = {}

                    def emit_qk(k):
                        d = k - 4 * qc
                        qlo = 128 * d if d > 0 else 0
                        for gg in range(2):
                            ps = pssp.tile([128, 2, CH], F32, tag=f"pss{gg}")
                            for hh in range(2):
                                hp = 64 * hh
                                nc.tensor.matmul(
                                    ps[:, hh, qlo:],
                                    lhsT=kt_sb[hp:hp + 64, gg, 128 * k:128 * (k + 1)],
                                    rhs=qt_sb[hp:hp + 64, gg, q0 + qlo:q0 + CH],
                                    start=True, stop=True, tile_position=(hp, 0))
                            pt = ptp.tile([128, 2, CH], BF16, tag="pt")
                            nc.scalar.activation(out=pt[:, :, qlo:], in_=ps[:, :, qlo:],
                                                 func=AF.Exp, scale=SCALE)
                            if d >= 0:
                                nc.gpsimd.tensor_tensor(
                                    out=pt[:, :, qlo:], in0=pt[:, :, qlo:],
                                    in1=masks_sb[:, d, :, qlo:], op=ALU.mult)
                            pts[(k, gg)] = pt

                    def emit_pv(k):
                        d = k - 4 * qc
                        qlo = 128 * d if d > 0 else 0
                        for gg in range(2):
                            pt = pts.pop((k, gg))
                            for hh in range(2):
                                nc.tensor.matmul(
                                    pa[gg][64 * hh:64 * (hh + 1), qlo:],
                                    lhsT=v_sb[:, k, 128 * gg + 64 * hh:128 * gg + 64 * (hh + 1)],
                                    rhs=pt[:, hh, qlo:],
                                    start=(k == 0), stop=(k == nkt - 1),
                                    tile_position=(0, 64 * hh),
                                    skip_group_check=(hh == 1))
                            for hh in range(2):
                                h = 2 * gg + hh
                                nc.tensor.matmul(
                                    pasum[32 * h:32 * h + 1, qlo:],
                                    lhsT=ones_1[:], rhs=pt[:, hh, qlo:],
                                    start=(k == 0), stop=(k == nkt - 1),
                                    tile_position=(0, 32 * h),
                                    skip_group_check=(h > 0))

                    for k in range(nkt + 3):
                        if k < nkt:
                            emit_qk(k)
                        if k >= 3:
                            emit_pv(k - 3)

                    recf = smp.tile([128, CH], F32, tag="recf")
                    nc.vector.reciprocal_approx_fast(out=recf[:], in_=pasum[:])
                    rec = smp.tile([128, CH], BF16, tag="rec")
                    nc.vector.tensor_scalar(out=rec[:], in0=recf[:], scalar1=1.0,
                                            scalar2=None, op0=ALU.mult)
                    for gg in range(2):
                        an = smp.tile([128, CH], BF16, tag=f"an{gg}")
                        nc.scalar.copy(out=an[:], in_=pa[gg][:])
                        rb = psr.tile([128, CH], F32, tag="recb")
                        for hh in range(2):
                            h = 2 * gg + hh
                            nc.tensor.matmul(
                                rb[64 * hh:64 * (hh + 1), :],
                                lhsT=ones64b[32 * h:32 * h + 1, :],
                                rhs=rec[32 * h:32 * h + 1, :],
                                start=True, stop=True,
                                tile_position=(32 * h, 64 * hh),
                                skip_group_check=(hh == 1))
                        at_t = smp.tile([128, CH], F8, tag=f"at{gg}")
                        nc.vector.tensor_tensor(out=at_t[:], in0=an[:], in1=rb[:],
                                                op=ALU.mult)
                        nc.sync.dma_start(out=a2a_in[2 * qc, gg],
                                          in_=at_t[:, 0:CH // 2])
                        nc.sync.dma_start(out=a2a_in[2 * qc + 1, gg],
                                          in_=at_t[:, CH // 2:CH])

            qkv_cm.__exit__(None, None, None)

            # ================= Phase 3: AllToAll (8 ranks, fp8 payload) ============
            nc.gpsimd.collective_compute(
                "AllToAll", ALU.bypass,
                replica_groups=[list(range(N_CORES))],
                ins=[a2a_in[:].opt()], outs=[a2a_out[:].opt()],
            )

            # ================= Phase 4: Wo + LN2 + FFN (feature-major) ============
            with (
                tc.tile_pool(name="ef", bufs=1) as ef,
                tc.tile_pool(name="efw", bufs=2) as efw,
                tc.tile_pool(name="psE", bufs=3, space="PSUM") as psE,
                tc.tile_pool(name="psES", bufs=1, space="PSUM") as psES,
                tc.tile_pool(name="w1p", bufs=4) as w1p,
                tc.tile_pool(name="w2p", bufs=3) as w2p,
            ):
                # keep-warm dummies riding over the collective
                dm = psES.tile([128, CH], F32, tag="dummy")
                for i in range(N_DUMMY):
                    nc.tensor.matmul(dm[:, 0:128], lhsT=ident[:],
                                     rhs=xself_sb[:, 0, 0:128],
                                     start=(i == 0), stop=(i == N_DUMMY - 1))

                # token axis of stage E: [0:256] = batch-0 strip, [256:512] = batch-1
                attnt = ef.tile([128, KT, CH], F8)
                for s in range(N_CORES):
                    bs, hgs = s // NG, s % NG
                    for g in range(2):
                        nc.sync.dma_start(
                            out=attnt[:, 2 * hgs + g,
                                      (CH // 2) * bs:(CH // 2) * (bs + 1)],
                            in_=a2a_out[s, g])

                # Wo (fp8 DoubleRow) + bo + residual
                x2 = ef.tile([128, KT, CH], BF16)
                for co in range(KT):
                    ps = psE.tile([128, CH], F32, tag="ps")
                    for t2 in range(KT // 2):
                        nc.tensor.matmul(ps[:],
                                         lhsT=wo_sb[:, 2 * t2:2 * t2 + 2, co, :],
                                         rhs=attnt[:, 2 * t2:2 * t2 + 2, :],
                                         start=(t2 == 0), stop=(t2 == KT // 2 - 1),
                                         perf_mode=DR)
                    prj = efw.tile([128, CH], BF16, tag="prj")
                    nc.scalar.activation(out=prj[:], in_=ps[:], func=AF.Identity,
                                         bias=bo_sb[:, co:co + 1], scale=ISW)
                    nc.vector.tensor_tensor(out=x2[:, co, :], in0=prj[:],
                                            in1=xself_sb[:, co, :], op=ALU.add)

                # LN2 (feature-major stats, E[x^2] form)
                st2 = psES.tile([128, CH], F32, tag="stat2")
                for k in range(KT):
                    nc.tensor.matmul(st2[0:1, :], lhsT=ones_c[:], rhs=x2[:, k, :],
                                     start=(k == 0), stop=(k == KT - 1))
                for k in range(KT):
                    sq2 = efw.tile([128, CH], BF16, tag="sq2", bufs=3)
                    nc.vector.tensor_tensor(out=sq2[:], in0=x2[:, k, :],
                                            in1=x2[:, k, :], op=ALU.mult)
                    nc.tensor.matmul(st2[32:33, :], lhsT=ones_c[:], rhs=sq2[:],
                                     start=(k == 0), stop=(k == KT - 1))
                mur2 = efw.tile([1, CH], BF16, tag="mur2")
                nc.scalar.copy(out=mur2[:], in_=st2[0:1, :])
                murf2 = efw.tile([1, CH], F32, tag="murf2")
                nc.scalar.copy(out=murf2[:], in_=st2[0:1, :])
                psb3 = psE.tile([128, CH], F32, tag="ps")
                nc.tensor.matmul(psb3[:], lhsT=onesrow[0:1, 0:128], rhs=mur2[:],
                                 start=True, stop=True)
                mub2 = efw.tile([128, CH], BF16, tag="mub2")
                nc.scalar.copy(out=mub2[:], in_=psb3[:])
                musq2 = efw.tile([1, CH], F32, tag="musq2")
                nc.vector.tensor_tensor(out=musq2[:], in0=murf2[:], in1=murf2[:],
                                        op=ALU.mult)
                varr2 = efw.tile([1, CH], F32, tag="varr2")
                nc.vector.tensor_tensor(out=varr2[:], in0=st2[32:33, :], in1=musq2[:],
                                        op=ALU.subtract)
                stdr2 = efw.tile([1, CH], F32, tag="stdr2")
                nc.scalar.activation(out=stdr2[:], in_=varr2[:], func=AF.Sqrt,
                                     bias=eps1[:], scale=1.0)
                rstdr2 = efw.tile([1, CH], BF16, tag="rstdr2")
                with nc.allow_low_precision(reason="ln2 rstd bf16"):
                    nc.vector.reciprocal(out=rstdr2[:], in_=stdr2[:])
                psb4 = psE.tile([128, CH], F32, tag="ps")
                nc.tensor.matmul(psb4[:], lhsT=onesrow[0:1, 0:128], rhs=rstdr2[:],
                                 start=True, stop=True)
                rstdb2 = efw.tile([128, CH], BF16, tag="rstdb2")
                nc.scalar.copy(out=rstdb2[:], in_=psb4[:])
                xc2 = ef.tile([128, KT, CH], BF16)
                for k in range(KT):
                    nc.vector.tensor_tensor(out=xc2[:, k, :], in0=x2[:, k, :],
                                            in1=mub2[:], op=ALU.subtract)
                h2t = ef.tile([128, KT, CH], F8)
                for k in range(KT):
                    nc.vector.tensor_tensor(out=h2t[:, k, :], in0=xc2[:, k, :],
                                            in1=rstdb2[:], op=ALU.mult)

                # FFN1 (fp8 DoubleRow), ReLU+bias+unscale fused on ACT
                hid = ef.tile([128, 32, CH], F8)
                for m in range(32):
                    w1t = w1p.tile([128, KT, 128], F8, tag="w1t")
                    nc.sync.dma_start(w1t[:], w1[:, :, 128 * m:128 * (m + 1)])
                    ps = psE.tile([128, CH], F32, tag="ps")
                    for t2 in range(KT // 2):
                        nc.tensor.matmul(ps[:], lhsT=w1t[:, 2 * t2:2 * t2 + 2, :],
                                         rhs=h2t[:, 2 * t2:2 * t2 + 2, :],
                                         start=(t2 == 0), stop=(t2 == KT // 2 - 1),
                                         perf_mode=DR)
                    nc.scalar.activation(out=hid[:, m, :], in_=ps[:], func=AF.Relu,
                                         bias=b1_sb[:, m:m + 1], scale=ISW)

                # FFN2 (fp8 DoubleRow) + b2 + residual
                for co in range(KT):
                    w2t = w2p.tile([128, 32, 128], F8, tag="w2t")
                    nc.sync.dma_start(w2t[:], w2[co])
                    ps = psE.tile([128, CH], F32, tag="ps")
                    for t2 in range(16):
                        nc.tensor.matmul(ps[:], lhsT=w2t[:, 2 * t2:2 * t2 + 2, :],
                                         rhs=hid[:, 2 * t2:2 * t2 + 2, :],
                                         start=(t2 == 0), stop=(t2 == 15),
                                         perf_mode=DR)
                    ft = efw.tile([128, CH], BF16, tag="ft")
                    nc.scalar.activation(out=ft[:], in_=ps[:], func=AF.Identity,
                                         bias=b2_sb[:, co:co + 1], scale=ISW2)
                    yt = efw.tile([128, CH], F32, tag="yt")
                    nc.vector.tensor_tensor(out=yt[:], in0=ft[:], in1=x2[:, co, :],
                                            op=ALU.add)
                    nc.sync.dma_start(y[co], yt[:])

    nc.compile()
    return nc


def prep_inputs(x, Wq, Wk, Wv, Wo, bo, W1, b1, W2, b2, g1, be1, g2, be2):
    """Host-side sharding / layout prep. Returns list of per-core input dicts."""
    bf = ml_dtypes.bfloat16
    f8 = mybir.dt.np(F8)
    x = np.asarray(x, np.float32)
    g1 = np.asarray(g1, np.float32); be1 = np.asarray(be1, np.float32)
    g2 = np.asarray(g2, np.float32); be2 = np.asarray(be2, np.float32)
    Wq = np.asarray(Wq, np.float32); Wk = np.asarray(Wk, np.float32)
    Wv = np.asarray(Wv, np.float32); Wo = np.asarray(Wo, np.float32)
    W1 = np.asarray(W1, np.float32); W2 = np.asarray(W2, np.float32)
    bo = np.asarray(bo, np.float32); b1 = np.asarray(b1, np.float32)
    b2 = np.asarray(b2, np.float32)

    Wq_f = g1[:, None] * Wq; bq_f = be1 @ Wq
    Wk_f = g1[:, None] * Wk; bk_f = be1 @ Wk
    Wv_f = g1[:, None] * Wv; bv_f = be1 @ Wv
    W1_f = g2[:, None] * W1; b1_f = b1 + be2 @ W1

    def lhsT_layout(w, scale):  # [C_in, M] -> [128, C_in//128, M] fp8
        ci, m = w.shape
        return np.ascontiguousarray(
            (w * scale).reshape(ci // 128, 128, m).transpose(1, 0, 2)).astype(f8)

    def tmajor(a):  # [rows, C] -> x^T tiles [128, KT, rows] bf16
        return np.ascontiguousarray(
            a.T.reshape(KT, 128, -1).transpose(1, 0, 2)).astype(bf)

    wo_l = np.ascontiguousarray(
        (Wo * SW).reshape(KT, 128, KT, 128).transpose(1, 0, 2, 3)).astype(f8)
    w1_l = lhsT_layout(W1_f, SW)
    w2_l = np.ascontiguousarray(
        (W2 * SW2).reshape(32, 128, KT, 128).transpose(2, 1, 0, 3)).astype(f8)
    b1_l = np.ascontiguousarray(b1_f.reshape(32, 128).T).astype(np.float32)
    b2_l = np.ascontiguousarray(b2.reshape(KT, 128).T).astype(np.float32)
    bo_l = np.ascontiguousarray(bo.reshape(KT, 128).T).astype(np.float32)

    # causal masks [p, d, hh, q]: valid iff 128*d + p <= q (hh dim replicated)
    m3 = np.zeros((128, 4, CH), np.float32)
    for d in range(4):
        kl = 128 * d + np.arange(128)[:, None]
        ql = np.arange(CH)[None, :]
        m3[:, d, :] = (kl <= ql).astype(np.float32)
    masks_l = np.ascontiguousarray(
        np.repeat(m3[:, :, None, :], 2, axis=2).transpose(0, 1, 2, 3)).astype(bf)

    ins = []
    for c in range(N_CORES):
        b = c // NG
        cols = slice(FPC * (c % NG), FPC * (c % NG + 1))
        xb = x[b]  # [T, C]
        strip = slice((CH // 2) * c, (CH // 2) * (c + 1))
        xown = np.concatenate([x[0][strip], x[1][strip]], axis=0)  # [CH, C]
        bq_c = bq_f[cols].reshape(2, 128).T
        bk_c = bk_f[cols].reshape(2, 128).T
        bv_c = bv_f[cols].reshape(2, 128).T
        ins.append({
            "xt": tmajor(xb),
            "xself": tmajor(xown),
            "wq": lhsT_layout(Wq_f[:, cols], SW),
            "wk": lhsT_layout(Wk_f[:, cols], SW),
            "wv": lhsT_layout(Wv_f[:, cols], SW),
            "bqkv": np.ascontiguousarray(
                np.stack([bq_c, bk_c, bv_c], axis=2)).astype(np.float32),
            "wo": wo_l, "bo_col": bo_l,
            "w1": w1_l, "b1": b1_l,
            "w2": w2_l, "b2col": b2_l,
            "masks": masks_l,
        })
    return ins


def postprocess(results):
    """Per-core y^T tiles [KT, 128, CH] -> full [B, T, C]."""
    out = np.empty((B, T, C), np.float32)
    hs = CH // 2
    for c, r in enumerate(results):
        yt = np.asarray(r["y"], np.float32).reshape(C, CH)  # [feats, toks]
        out[0, hs * c:hs * (c + 1), :] = yt[:, 0:hs].T
        out[1, hs * c:hs * (c + 1), :] = yt[:, hs:CH].T
    return out


_NC_CACHE = {}


def kernel(**inputs):
    import time
    from concourse.bass_utils import run_bass_kernel_spmd
    if "nc" not in _NC_CACHE:
        _NC_CACHE["nc"] = build_nc()
    nc = _NC_CACHE["nc"]
    ins = prep_inputs(**inputs)
    res = None
    last_exc = None
    for _attempt in range(4):
        try:
            res = run_bass_kernel_spmd(nc, ins, core_ids=list(range(N_CORES)))
            break
        except Exception as e:  # transient device wedge
            last_exc = e
            time.sleep(2)
    if res is None:
        raise last_exc
    return postprocess(res.results)
